# revision 13
# baseline (speedup 1.0000x reference)
"""Additive attention kernel for Trainium2, 8 NeuronCores (SPMD).

Reference computation (B=4, Q=128, K=1024, H=256, QS=KS=DV=256):
    q = queries @ Wq                    [B,Q,H]
    k = keys @ Wk                       [B,K,H]
    feats = tanh(q[:,:,None,:] + k[:,None,:,:])
    scores = feats @ Wv                 [B,Q,K]
    masked softmax over K (valid_lens), out = attn @ values   [B,Q,DV]

Sharding: Q is split across the 8 cores (16 q-rows per core, every core
processes all 4 batches) -- perfectly balanced, no collectives.  The
kernel is specialized at trace time on the runtime valid_lens values, so
only valid key positions are ever computed.

Per-core schedule (the ACT engine's ~43us of tanh+exp is the hard
floor; everything else is arranged so ACT never waits):
  DMA: three queues -- (Wq|Wk|qT) first, then keysT in batch processing
    order + wz, then identity + values.  Batches are processed smallest
    VL first (fast pipeline fill), largest next, a small one last
    (short drain).
  PE: qp/kp projections (bf16, fp32 PSUM), then per-q one-hot Wv score
    matmuls striped over the four PE column groups as before.
  DVE: PSUM->SBUF copies and the per-q broadcast adds
    (kp + qp column -> bf16 slab covering BOTH 128-row h-chunks).
    Emission is interleaved so batch i+1's kp copies / PSUM memsets
    never sit between batch i's adds in the DVE queue.
  ACT: one Tanh per slab (both h-chunks at once), graduated slab sizes
    [2,2,4,8] first batch / [8,4,2,2] last.  Each batch's Exp is
    deferred until after the NEXT batch's first tanh slab, so ACT never
    stalls on PE draining the score matmuls.  Tanh and Exp share one
    activation table (exp_and_others) -> no table reloads.
  Softmax + AV per batch: exp off the PSUM stripes, PE bf16 transpose,
    DVE compaction of the 16 live rows, AV matmul against
    ones-augmented values (denominator in the extra column),
    reciprocal + scale, DMA out.  The 4-way q striping permutation is
    undone on the host after gather.
"""

import numpy as np

B, Q, KMAX, H = 4, 128, 1024, 256
QS, KS, DV = 256, 256, 256
N_CORES = 8
QSH = Q // N_CORES  # 16 q rows per core

# q index permutation induced by the 4-way PSUM striping (self-inverse)
PERM = np.array([4 * (c % 4) + c // 4 for c in range(QSH)])

_PROGRAM_CACHE: dict = {}


def _even(x: int) -> int:
    return x + (x & 1)


def _windows(total: int, step: int):
    out = []
    s = 0
    while s < total:
        out.append((s, min(step, total - s)))
        s += step
    return out


def _proc_order(vl):
    """Smallest batch first (fast fill), then the rest descending
    (big batches mid-stream, a small one last for a short drain)."""
    order = sorted(range(B), key=lambda b: vl[b])
    return [order[0]] + order[1:][::-1]


def _slab_plan(pi: int) -> list:
    if pi == 0:
        return [2, 2, 4, 8]
    if pi == B - 1:
        return [8, 4, 2, 2]
    return [8, 8]


def _geom(vl):
    """Shared geometry: valid lens, paddings, blob column offsets."""
    VL = list(vl)
    VLP = [_even(v) for v in VL]
    PROC = _proc_order(VL)
    NCH = [(v + 127) // 128 for v in VL]
    # keysT packing windows span the even-padded length; score/exp
    # windows span the true valid length (the pad column never gets a
    # score).  The window COUNTS always match.
    WK = {b: _windows(VLP[b], 512) for b in range(B)}
    WS = {b: _windows(VL[b], 512) for b in range(B)}
    # bf16 blob column layout, in DMA arrival order
    cols = {}
    c = 0
    # --- queue 0 (sync, two starts): [wq|qT] gates qp, [wk|keysT of
    #     the first batch] gates the first kp ---
    for e in range(2):
        cols[f"wq{e}"] = c
        c += H
    for e in range(2):
        cols[f"qT{e}"] = c
        c += B * QSH
    cols["q0b"] = c
    for e in range(2):
        cols[f"wk{e}"] = c
        c += H
    for wi in range(len(WK[PROC[0]])):
        for e in range(2):
            cols[f"keysT_b{PROC[0]}_w{wi}_e{e}"] = c
            c += WK[PROC[0]][wi][1]
    cols["q1"] = c
    # --- queue 1 (vector): second batch keysT, wz, remaining keysT ---
    for wi in range(len(WK[PROC[1]])):
        for e in range(2):
            cols[f"keysT_b{PROC[1]}_w{wi}_e{e}"] = c
            c += WK[PROC[1]][wi][1]
    for hc in range(2):
        cols[f"wz{hc}"] = c
        c += QSH + 31  # sliding one-hot windows of width 32, Wv at QSH-1
    for b in PROC[2:]:
        for wi in range(len(WK[b])):
            for e in range(2):
                cols[f"keysT_b{b}_w{wi}_e{e}"] = c
                c += WK[b][wi][1]
    cols["q2"] = c
    # --- queue 2 (gpsimd): identity + values (chunk order = PROC) ---
    cols["identb"] = c
    c += 128
    slot = 0
    for b in PROC:
        for ci in range(NCH[b]):
            cols[f"vals_b{b}_c{ci}"] = c + slot * (DV + 1)
            slot += 1
    c += slot * (DV + 1)
    CW = c
    return VL, VLP, PROC, NCH, WK, WS, cols, CW


def _build_program(vl: tuple):
    import concourse.bacc as bacc
    import concourse.mybir as mybir
    import concourse.tile as tile

    dt = mybir.dt
    AF = mybir.ActivationFunctionType
    VL, VLP, PROC, NCH, WK, WS, cols, CW = _geom(vl)

    nc = bacc.Bacc("TRN2", target_bir_lowering=False, debug=False,
                   num_devices=N_CORES)

    d_blob = nc.dram_tensor("blob", [128, CW], dt.bfloat16,
                            kind="ExternalInput")
    d_out = nc.dram_tensor("out", [B, QSH, DV], dt.float32,
                           kind="ExternalOutput")

    with tile.TileContext(nc) as tc:
        with (
            tc.tile_pool(name="const", bufs=1) as constp,
            tc.tile_pool(name="kp", bufs=1) as kpp,
            tc.tile_pool(name="qp", bufs=2) as qpp,
            tc.tile_pool(name="pre", bufs=3) as prep,
            tc.tile_pool(name="feats", bufs=3) as featsp,
            tc.tile_pool(name="scsb", bufs=2) as scsbp,
            tc.tile_pool(name="expt", bufs=4) as exptp,
            tc.tile_pool(name="outsb", bufs=2) as outsbp,
            tc.tile_pool(name="rcp", bufs=2) as rcpp,
            tc.tile_pool(name="pswork", bufs=3, space="PSUM") as pswork,
            tc.tile_pool(name="pssc", bufs=4, space="PSUM") as pssc,
            tc.tile_pool(name="psav", bufs=1, space="PSUM") as psav,
        ):
            blob = constp.tile([128, CW], dt.bfloat16, tag="blob")
            c0b, c1, c2 = cols["q0b"], cols["q1"], cols["q2"]
            nc.sync.dma_start(out=blob[:, 0:c0b], in_=d_blob[:, 0:c0b])
            nc.sync.dma_start(out=blob[:, c0b:c1], in_=d_blob[:, c0b:c1])
            # the big streams go on slower-starting queues (gpsimd SWDGE /
            # ACT), giving the critical sync queue HBM bandwidth first
            nc.gpsimd.dma_start(out=blob[:, c1:c2], in_=d_blob[:, c1:c2])
            nc.scalar.dma_start(out=blob[:, c2:CW], in_=d_blob[:, c2:CW])

            def bl(name, width):
                c0 = cols[name]
                return blob[:, c0:c0 + width]

            # ---- q projection ----
            qp = [None, None]  # [hc] -> [128, B*QSH] f32
            for hc in range(2):
                hs = slice(hc * 128, hc * 128 + 128)
                ps = pswork.tile([128, B * QSH], dt.float32, tag="w",
                                 name=f"psq{hc}")
                nc.tensor.matmul(ps[:], bl("wq0", H)[:, hs],
                                 bl("qT0", B * QSH), start=True, stop=False)
                nc.tensor.matmul(ps[:], bl("wq1", H)[:, hs],
                                 bl("qT1", B * QSH), start=False, stop=True)
                qp[hc] = qpp.tile([128, B * QSH], dt.float32, tag="qp",
                                  name=f"qp{hc}")
                nc.vector.tensor_copy(qp[hc][:], ps[:])

            # ---- k projections: PE matmuls all up front (PROC order);
            #      the PSUM->SBUF copies are interleaved into the batch
            #      loop so they never delay the adds gating a tanh ----
            kp = {b: kpp.tile([128, 2 * VLP[b]], dt.bfloat16,
                              tag=f"kp{b}", name=f"kp{b}")
                  for b in range(B)}
            kp_ps = {}  # (b, wi, hc) -> psum tile
            for b in PROC:
                for wi, (w0, wl) in enumerate(WK[b]):
                    for hc in range(2):
                        hs = slice(hc * 128, hc * 128 + 128)
                        ps2 = pswork.tile([128, wl], dt.float32, tag="w",
                                          name=f"psk{b}_{wi}_{hc}")
                        nc.tensor.matmul(ps2[:], bl("wk0", H)[:, hs],
                                         bl(f"keysT_b{b}_w{wi}_e0", wl),
                                         start=True, stop=False)
                        nc.tensor.matmul(ps2[:], bl("wk1", H)[:, hs],
                                         bl(f"keysT_b{b}_w{wi}_e1", wl),
                                         start=False, stop=True)
                        kp_ps[(b, wi, hc)] = ps2

            def emit_kp_copies(b, on_act=False):
                for wi, (w0, wl) in enumerate(WK[b]):
                    for hc in range(2):
                        dst = kp[b][:, hc * VLP[b] + w0:
                                    hc * VLP[b] + w0 + wl]
                        src_ps = kp_ps.pop((b, wi, hc))[:]
                        if on_act:
                            # ACT idles during the first batch's tiny
                            # slabs; Copy shares the Tanh/Exp act table
                            nc.scalar.copy(dst, src_ps)
                        else:
                            nc.vector.tensor_copy(dst, src_ps)

            emit_kp_copies(PROC[0])

            # sc PSUM banks: allocated + zeroed (DVE, width wl only --
            # exp/matmuls never touch columns past wl) one batch ahead
            sc_banks = {}

            def prep_banks(b):
                sc_banks[b] = [pssc.tile([128, 512], dt.float32, tag="sc",
                                         name=f"sc{b}_{wi}")
                               for wi in range(len(WS[b]))]
                for wi, (w0, wl) in enumerate(WS[b]):
                    nc.vector.memset(sc_banks[b][wi][:, 0:wl], 0.0)

            prep_banks(PROC[0])

            # ---- deferred per-batch softmax + AV tail ----
            def emit_post(b):
                vlb, vlpb = VL[b], VLP[b]
                sc_ps = sc_banks[b]
                exp_sb = scsbp.tile([128, vlb], dt.bfloat16, tag="scsb",
                                    name=f"scsb{b}")
                for wi, (w0, wl) in enumerate(WS[b]):
                    nc.scalar.activation(exp_sb[:, w0:w0 + wl],
                                         sc_ps[wi][:, 0:wl], AF.Exp)
                av = psav.tile([QSH, DV + 1], dt.float32, tag="av",
                               name=f"av{b}")
                for ci in range(NCH[b]):
                    c0 = ci * 128
                    csz = min(128, vlb - c0)
                    trp = pswork.tile([csz, 128], dt.bfloat16, tag="w",
                                      name=f"tr{b}_{ci}")
                    nc.tensor.transpose(trp[:], exp_sb[:, c0:c0 + csz],
                                        bl("identb", 128))
                    ex = exptp.tile([csz, QSH], dt.bfloat16, tag="ex",
                                    name=f"ex{b}_{ci}")
                    nc.vector.tensor_copy(
                        ex[:].rearrange("p (s r) -> p s r", s=4),
                        trp[:].rearrange("p (s x r) -> p s x r",
                                         s=4, x=8)[:, :, 0, 0:4])
                    vcol = cols[f"vals_b{b}_c{ci}"]
                    nc.tensor.matmul(av[:], ex[:],
                                     blob[0:csz, vcol:vcol + DV + 1],
                                     start=(ci == 0),
                                     stop=(ci == NCH[b] - 1))
                rc = rcpp.tile([QSH, 1], dt.float32, tag="rc",
                               name=f"rc{b}")
                nc.vector.reciprocal(rc[:], av[:, DV:DV + 1])
                ob = outsbp.tile([QSH, DV], dt.float32, tag="ob",
                                 name=f"ob{b}")
                nc.vector.tensor_scalar_mul(ob[:], av[:, 0:DV], rc[:])
                nc.sync.dma_start(out=d_out[b], in_=ob[:])

            # ---- main loop: adds (DVE) -> tanh (ACT) -> score matmuls
            #      (PE), with deferred exp/AV for the previous batch ----
            for pi, b in enumerate(PROC):
                vlb, vlpb = VL[b], VLP[b]
                plan = _slab_plan(pi)
                q0 = 0
                for si, g in enumerate(plan):
                    pre_t = prep.tile([128, g * 2 * vlpb], dt.bfloat16,
                                      tag="pre", name=f"pre{b}_{si}")
                    for j in range(g):
                        for hc in range(2):
                            nc.vector.tensor_scalar_add(
                                pre_t[:, (2 * j + hc) * vlpb:
                                      (2 * j + hc + 1) * vlpb],
                                kp[b][:, hc * vlpb:(hc + 1) * vlpb],
                                qp[hc][:, b * QSH + q0 + j:
                                       b * QSH + q0 + j + 1])
                    feats_t = featsp.tile([128, g * 2 * vlpb], dt.bfloat16,
                                          tag="feats", name=f"ft{b}_{si}")
                    nc.scalar.activation(feats_t[:], pre_t[:], AF.Tanh)
                    if si == 0 and pi > 0:
                        emit_post(PROC[pi - 1])
                    if si == 1 and pi + 1 < len(PROC):
                        emit_kp_copies(PROC[pi + 1], on_act=(pi == 0))
                        prep_banks(PROC[pi + 1])
                    for j in range(g):
                        qq = q0 + j
                        s, r = qq % 4, qq // 4
                        first = (si == 0 and j == 0)
                        last = (si == len(plan) - 1 and j == g - 1)
                        for hc in range(2):
                            lhsT = bl(f"wz{hc}", QSH + 31)[
                                :, QSH - 1 - r: QSH + 31 - r]
                            for wi, (w0, wl) in enumerate(WS[b]):
                                nc.tensor.matmul(
                                    sc_banks[b][wi][32 * s:32 * s + 32, 0:wl],
                                    lhsT,
                                    feats_t[:, (2 * j + hc) * vlpb + w0:
                                            (2 * j + hc) * vlpb + w0 + wl],
                                    start=(first and hc == 0),
                                    stop=(last and hc == 1),
                                    tile_position=(0, 32 * s),
                                    skip_group_check=True)
                    q0 += g
            emit_post(PROC[-1])

    nc.compile()
    return nc


def _host_prep(queries, keys, values, vl, Wq, Wk, Wv):
    """Build the 8 per-core input maps (slicing / transposes / packing)."""
    import ml_dtypes
    bf16 = ml_dtypes.bfloat16

    queries = np.ascontiguousarray(np.asarray(queries, np.float32))
    keys = np.asarray(keys, np.float32)
    values = np.asarray(values, np.float32)
    Wq = np.asarray(Wq, np.float32)
    Wk = np.asarray(Wk, np.float32)
    Wv = np.asarray(Wv, np.float32)

    VL, VLP, PROC, NCH, WK, WS, cols, CW = _geom(vl)

    shared = np.zeros((128, CW), np.float32)
    for e in range(2):
        shared[:, cols[f"wq{e}"]:cols[f"wq{e}"] + H] = \
            Wq[e * 128:(e + 1) * 128, :]
        shared[:, cols[f"wk{e}"]:cols[f"wk{e}"] + H] = \
            Wk[e * 128:(e + 1) * 128, :]
    for b in range(B):
        kT = np.zeros((2, 128, VLP[b]), np.float32)
        kT[:, :, :VL[b]] = keys[b, :VL[b], :].T.reshape(2, 128, VL[b])
        for wi, (w0, wl) in enumerate(WK[b]):
            for e in range(2):
                c0 = cols[f"keysT_b{b}_w{wi}_e{e}"]
                shared[:, c0:c0 + wl] = kT[e, :, w0:w0 + wl]
    for hc in range(2):
        shared[:, cols[f"wz{hc}"] + QSH - 1] = Wv[hc * 128:(hc + 1) * 128]
    for b in range(B):
        for ci in range(NCH[b]):
            c0 = ci * 128
            csz = min(128, VL[b] - c0)
            vcol = cols[f"vals_b{b}_c{ci}"]
            shared[:csz, vcol:vcol + DV] = values[b, c0:c0 + csz, :]
            shared[:csz, vcol + DV] = 1.0
    shared[:, cols["identb"]:cols["identb"] + 128] = np.eye(128)

    in_maps = []
    for c in range(N_CORES):
        blob = shared.copy()
        qsl = queries[:, c * QSH:(c + 1) * QSH, :]  # [B, 16, QS]
        qT = np.ascontiguousarray(qsl.transpose(0, 2, 1))  # [B, QS, 16]
        for e in range(2):
            c0 = cols[f"qT{e}"]
            for b in range(B):
                blob[:, c0 + b * QSH:c0 + (b + 1) * QSH] = \
                    qT[b, e * 128:(e + 1) * 128, :]
        in_maps.append({"blob": blob.astype(bf16)})
    return in_maps


def kernel(queries, keys, values, valid_lens, Wq, Wk, Wv):
    from concourse.bass_utils import run_bass_kernel_spmd

    vl = tuple(int(x) for x in np.asarray(valid_lens).reshape(-1))
    assert len(vl) == B and all(1 <= v <= KMAX for v in vl)

    if vl not in _PROGRAM_CACHE:
        _PROGRAM_CACHE[vl] = _build_program(vl)
    nc = _PROGRAM_CACHE[vl]

    in_maps = _host_prep(queries, keys, values, vl, Wq, Wk, Wv)
    res = run_bass_kernel_spmd(nc, in_maps, list(range(N_CORES)))

    out = np.empty((B, Q, DV), np.float32)
    for c in range(N_CORES):
        # device rows are in PERM order: row c holds q = PERM[c]
        out[:, c * QSH + PERM, :] = res.results[c]["out"]
    return out


# revision 15
# speedup vs baseline: 1.2165x; 1.2165x over previous
"""Additive attention kernel for Trainium2, 8 NeuronCores (SPMD).

Reference computation (B=4, Q=128, K=1024, H=256, QS=KS=DV=256):
    q = queries @ Wq                    [B,Q,H]
    k = keys @ Wk                       [B,K,H]
    feats = tanh(q[:,:,None,:] + k[:,None,:,:])
    scores = feats @ Wv                 [B,Q,K]
    masked softmax over K (valid_lens), out = attn @ values   [B,Q,DV]

Sharding: Q is split across the 8 cores (16 q-rows per core, every core
processes all 4 batches) -- perfectly balanced, no collectives.  The
kernel is specialized at trace time on the runtime valid_lens values, so
only valid key positions are ever computed.

Per-core schedule (the ACT engine's ~43us of tanh+exp is the hard
floor; everything else is arranged so ACT never waits):
  DMA: three queues -- (Wq|Wk|qT) first, then keysT in batch processing
    order + wz, then identity + values.  Batches are processed smallest
    VL first (fast pipeline fill), largest next, a small one last
    (short drain).
  PE: qp/kp projections (bf16, fp32 PSUM), then per-q one-hot Wv score
    matmuls striped over the four PE column groups as before.
  DVE: PSUM->SBUF copies and the per-q broadcast adds
    (kp + qp column -> bf16 slab covering BOTH 128-row h-chunks).
    Emission is interleaved so batch i+1's kp copies / PSUM memsets
    never sit between batch i's adds in the DVE queue.
  ACT: one Tanh per slab (both h-chunks at once), graduated slab sizes
    [2,2,4,8] first batch / [8,4,2,2] last.  Each batch's Exp is
    deferred until after the NEXT batch's first tanh slab, so ACT never
    stalls on PE draining the score matmuls.  Tanh and Exp share one
    activation table (exp_and_others) -> no table reloads.
  Softmax + AV per batch: exp off the PSUM stripes, PE bf16 transpose,
    DVE compaction of the 16 live rows, AV matmul against
    ones-augmented values (denominator in the extra column),
    reciprocal + scale, DMA out.  The 4-way q striping permutation is
    undone on the host after gather.
"""

import numpy as np

B, Q, KMAX, H = 4, 128, 1024, 256
QS, KS, DV = 256, 256, 256
N_CORES = 8
QSH = Q // N_CORES  # 16 q rows per core

# q index permutation induced by the 4-way PSUM striping (self-inverse)
PERM = np.array([4 * (c % 4) + c // 4 for c in range(QSH)])

_PROGRAM_CACHE: dict = {}


def _even(x: int) -> int:
    return x + (x & 1)


def _windows(total: int, step: int):
    out = []
    s = 0
    while s < total:
        out.append((s, min(step, total - s)))
        s += step
    return out


def _proc_order(vl):
    """Smallest batch first (fast fill), then the rest descending
    (big batches mid-stream, a small one last for a short drain)."""
    order = sorted(range(B), key=lambda b: vl[b])
    return [order[0]] + order[1:][::-1]


def _slab_plan(pi: int) -> list:
    if pi == 0:
        return [2, 2, 4, 8]
    if pi == B - 1:
        return [8, 4, 2, 2]
    return [8, 8]


def _geom(vl):
    """Shared geometry: valid lens, paddings, blob column offsets."""
    VL = list(vl)
    VLP = [_even(v) for v in VL]
    PROC = _proc_order(VL)
    NCH = [(v + 127) // 128 for v in VL]
    # keysT packing windows span the even-padded length; score/exp
    # windows span the true valid length (the pad column never gets a
    # score).  The window COUNTS always match.
    WK = {b: _windows(VLP[b], 512) for b in range(B)}
    WS = {b: _windows(VL[b], 512) for b in range(B)}
    # bf16 blob column layout, in DMA arrival order
    cols = {}
    c = 0
    # --- queue 0 (sync, two starts): [wq|qT] gates qp, [wk|keysT of
    #     the first batch] gates the first kp ---
    for e in range(2):
        cols[f"wq{e}"] = c
        c += H
    for e in range(2):
        cols[f"qT{e}"] = c
        c += B * QSH
    cols["q0b"] = c
    for e in range(2):
        cols[f"wk{e}"] = c
        c += H
    for wi in range(len(WK[PROC[0]])):
        for e in range(2):
            cols[f"keysT_b{PROC[0]}_w{wi}_e{e}"] = c
            c += WK[PROC[0]][wi][1]
    cols["q1"] = c
    # --- queue 1 (vector): second batch keysT, wz, remaining keysT ---
    for wi in range(len(WK[PROC[1]])):
        for e in range(2):
            cols[f"keysT_b{PROC[1]}_w{wi}_e{e}"] = c
            c += WK[PROC[1]][wi][1]
    for hc in range(2):
        cols[f"wz{hc}"] = c
        c += QSH + 31  # sliding one-hot windows of width 32, Wv at QSH-1
    for b in PROC[2:]:
        for wi in range(len(WK[b])):
            for e in range(2):
                cols[f"keysT_b{b}_w{wi}_e{e}"] = c
                c += WK[b][wi][1]
    cols["q2"] = c
    # --- queue 2 (gpsimd): identity + values (chunk order = PROC) ---
    cols["identb"] = c
    c += 128
    slot = 0
    for b in PROC:
        for ci in range(NCH[b]):
            cols[f"vals_b{b}_c{ci}"] = c + slot * (DV + 1)
            slot += 1
    c += slot * (DV + 1)
    CW = c
    return VL, VLP, PROC, NCH, WK, WS, cols, CW


def _build_program(vl: tuple):
    import concourse.bacc as bacc
    import concourse.mybir as mybir
    import concourse.tile as tile

    dt = mybir.dt
    AF = mybir.ActivationFunctionType
    VL, VLP, PROC, NCH, WK, WS, cols, CW = _geom(vl)

    nc = bacc.Bacc("TRN2", target_bir_lowering=False, debug=False,
                   num_devices=N_CORES)

    d_blob = nc.dram_tensor("blob", [128, CW], dt.bfloat16,
                            kind="ExternalInput")
    d_out = nc.dram_tensor("out", [B, QSH, DV], dt.float32,
                           kind="ExternalOutput")

    with tile.TileContext(nc) as tc:
        with (
            tc.tile_pool(name="const", bufs=1) as constp,
            tc.tile_pool(name="kp", bufs=1) as kpp,
            tc.tile_pool(name="qp", bufs=2) as qpp,
            tc.tile_pool(name="pre", bufs=3) as prep,
            tc.tile_pool(name="feats", bufs=3) as featsp,
            tc.tile_pool(name="scsb", bufs=2) as scsbp,
            tc.tile_pool(name="expt", bufs=4) as exptp,
            tc.tile_pool(name="outsb", bufs=2) as outsbp,
            tc.tile_pool(name="rcp", bufs=2) as rcpp,
            tc.tile_pool(name="pswork", bufs=3, space="PSUM") as pswork,
            tc.tile_pool(name="pssc", bufs=4, space="PSUM") as pssc,
            tc.tile_pool(name="psav", bufs=1, space="PSUM") as psav,
        ):
            blob = constp.tile([128, CW], dt.bfloat16, tag="blob")
            c0b, c1, c2 = cols["q0b"], cols["q1"], cols["q2"]
            # ONE SP queue, four starts: a single queue drains strictly
            # in order at full bandwidth -- the critical prefix ([wq|qT],
            # then [wk|keysT of the first batch]) lands first, the bulk
            # streams behind it, each signalling its own semaphore.
            nc.sync.dma_start(out=blob[:, 0:c0b], in_=d_blob[:, 0:c0b])
            nc.sync.dma_start(out=blob[:, c0b:c1], in_=d_blob[:, c0b:c1])
            nc.sync.dma_start(out=blob[:, c1:c2], in_=d_blob[:, c1:c2])
            nc.sync.dma_start(out=blob[:, c2:CW], in_=d_blob[:, c2:CW])

            def bl(name, width):
                c0 = cols[name]
                return blob[:, c0:c0 + width]

            # ---- q projection ----
            qp = [None, None]  # [hc] -> [128, B*QSH] f32
            for hc in range(2):
                hs = slice(hc * 128, hc * 128 + 128)
                ps = pswork.tile([128, B * QSH], dt.float32, tag="w",
                                 name=f"psq{hc}")
                nc.tensor.matmul(ps[:], bl("wq0", H)[:, hs],
                                 bl("qT0", B * QSH), start=True, stop=False)
                nc.tensor.matmul(ps[:], bl("wq1", H)[:, hs],
                                 bl("qT1", B * QSH), start=False, stop=True)
                qp[hc] = qpp.tile([128, B * QSH], dt.float32, tag="qp",
                                  name=f"qp{hc}")
                nc.vector.tensor_copy(qp[hc][:], ps[:])

            # ---- k projections: PE matmuls all up front (PROC order);
            #      the PSUM->SBUF copies are interleaved into the batch
            #      loop so they never delay the adds gating a tanh ----
            kp = {b: kpp.tile([128, 2 * VLP[b]], dt.bfloat16,
                              tag=f"kp{b}", name=f"kp{b}")
                  for b in range(B)}
            kp_ps = {}  # (b, wi, hc) -> psum tile
            for b in PROC:
                for wi, (w0, wl) in enumerate(WK[b]):
                    for hc in range(2):
                        hs = slice(hc * 128, hc * 128 + 128)
                        ps2 = pswork.tile([128, wl], dt.float32, tag="w",
                                          name=f"psk{b}_{wi}_{hc}")
                        nc.tensor.matmul(ps2[:], bl("wk0", H)[:, hs],
                                         bl(f"keysT_b{b}_w{wi}_e0", wl),
                                         start=True, stop=False)
                        nc.tensor.matmul(ps2[:], bl("wk1", H)[:, hs],
                                         bl(f"keysT_b{b}_w{wi}_e1", wl),
                                         start=False, stop=True)
                        kp_ps[(b, wi, hc)] = ps2

            def emit_kp_copies(b, on_act=False):
                for wi, (w0, wl) in enumerate(WK[b]):
                    for hc in range(2):
                        dst = kp[b][:, hc * VLP[b] + w0:
                                    hc * VLP[b] + w0 + wl]
                        src_ps = kp_ps.pop((b, wi, hc))[:]
                        nc.vector.tensor_copy(dst, src_ps)

            emit_kp_copies(PROC[0])

            # sc PSUM banks: allocated + zeroed (DVE, width wl only --
            # exp/matmuls never touch columns past wl) one batch ahead
            sc_banks = {}

            def prep_banks(b):
                sc_banks[b] = [pssc.tile([128, 512], dt.float32, tag="sc",
                                         name=f"sc{b}_{wi}")
                               for wi in range(len(WS[b]))]
                for wi, (w0, wl) in enumerate(WS[b]):
                    nc.vector.memset(sc_banks[b][wi][:, 0:wl], 0.0)

            prep_banks(PROC[0])

            # ---- deferred per-batch softmax + AV tail ----
            def emit_post(b):
                vlb, vlpb = VL[b], VLP[b]
                sc_ps = sc_banks[b]
                exp_sb = scsbp.tile([128, vlb], dt.bfloat16, tag="scsb",
                                    name=f"scsb{b}")
                for wi, (w0, wl) in enumerate(WS[b]):
                    nc.scalar.activation(exp_sb[:, w0:w0 + wl],
                                         sc_ps[wi][:, 0:wl], AF.Exp)
                av = psav.tile([QSH, DV + 1], dt.float32, tag="av",
                               name=f"av{b}")
                for ci in range(NCH[b]):
                    c0 = ci * 128
                    csz = min(128, vlb - c0)
                    trp = pswork.tile([csz, 128], dt.bfloat16, tag="w",
                                      name=f"tr{b}_{ci}")
                    nc.tensor.transpose(trp[:], exp_sb[:, c0:c0 + csz],
                                        bl("identb", 128))
                    ex = exptp.tile([csz, QSH], dt.bfloat16, tag="ex",
                                    name=f"ex{b}_{ci}")
                    nc.vector.tensor_copy(
                        ex[:].rearrange("p (s r) -> p s r", s=4),
                        trp[:].rearrange("p (s x r) -> p s x r",
                                         s=4, x=8)[:, :, 0, 0:4])
                    vcol = cols[f"vals_b{b}_c{ci}"]
                    nc.tensor.matmul(av[:], ex[:],
                                     blob[0:csz, vcol:vcol + DV + 1],
                                     start=(ci == 0),
                                     stop=(ci == NCH[b] - 1))
                rc = rcpp.tile([QSH, 1], dt.float32, tag="rc",
                               name=f"rc{b}")
                nc.vector.reciprocal(rc[:], av[:, DV:DV + 1])
                ob = outsbp.tile([QSH, DV], dt.float32, tag="ob",
                                 name=f"ob{b}")
                nc.vector.tensor_scalar_mul(ob[:], av[:, 0:DV], rc[:])
                nc.sync.dma_start(out=d_out[b], in_=ob[:])

            # ---- main loop: adds (DVE) -> tanh (ACT) -> score matmuls
            #      (PE), with deferred exp/AV for the previous batch ----
            for pi, b in enumerate(PROC):
                vlb, vlpb = VL[b], VLP[b]
                plan = _slab_plan(pi)
                q0 = 0
                for si, g in enumerate(plan):
                    pre_t = prep.tile([128, g * 2 * vlpb], dt.bfloat16,
                                      tag="pre", name=f"pre{b}_{si}")
                    for j in range(g):
                        for hc in range(2):
                            nc.vector.tensor_scalar_add(
                                pre_t[:, (2 * j + hc) * vlpb:
                                      (2 * j + hc + 1) * vlpb],
                                kp[b][:, hc * vlpb:(hc + 1) * vlpb],
                                qp[hc][:, b * QSH + q0 + j:
                                       b * QSH + q0 + j + 1])
                    feats_t = featsp.tile([128, g * 2 * vlpb], dt.bfloat16,
                                          tag="feats", name=f"ft{b}_{si}")
                    nc.scalar.activation(feats_t[:], pre_t[:], AF.Tanh)
                    if si == 0 and pi > 0:
                        emit_post(PROC[pi - 1])
                    if si == 1 and pi + 1 < len(PROC):
                        emit_kp_copies(PROC[pi + 1], on_act=(pi == 0))
                        prep_banks(PROC[pi + 1])
                    for j in range(g):
                        qq = q0 + j
                        s, r = qq % 4, qq // 4
                        first = (si == 0 and j == 0)
                        last = (si == len(plan) - 1 and j == g - 1)
                        for hc in range(2):
                            lhsT = bl(f"wz{hc}", QSH + 31)[
                                :, QSH - 1 - r: QSH + 31 - r]
                            for wi, (w0, wl) in enumerate(WS[b]):
                                nc.tensor.matmul(
                                    sc_banks[b][wi][32 * s:32 * s + 32, 0:wl],
                                    lhsT,
                                    feats_t[:, (2 * j + hc) * vlpb + w0:
                                            (2 * j + hc) * vlpb + w0 + wl],
                                    start=(first and hc == 0),
                                    stop=(last and hc == 1),
                                    tile_position=(0, 32 * s),
                                    skip_group_check=True)
                    q0 += g
            emit_post(PROC[-1])

    nc.compile()
    return nc


def _host_prep(queries, keys, values, vl, Wq, Wk, Wv):
    """Build the 8 per-core input maps (slicing / transposes / packing)."""
    import ml_dtypes
    bf16 = ml_dtypes.bfloat16

    queries = np.ascontiguousarray(np.asarray(queries, np.float32))
    keys = np.asarray(keys, np.float32)
    values = np.asarray(values, np.float32)
    Wq = np.asarray(Wq, np.float32)
    Wk = np.asarray(Wk, np.float32)
    Wv = np.asarray(Wv, np.float32)

    VL, VLP, PROC, NCH, WK, WS, cols, CW = _geom(vl)

    shared = np.zeros((128, CW), np.float32)
    for e in range(2):
        shared[:, cols[f"wq{e}"]:cols[f"wq{e}"] + H] = \
            Wq[e * 128:(e + 1) * 128, :]
        shared[:, cols[f"wk{e}"]:cols[f"wk{e}"] + H] = \
            Wk[e * 128:(e + 1) * 128, :]
    for b in range(B):
        kT = np.zeros((2, 128, VLP[b]), np.float32)
        kT[:, :, :VL[b]] = keys[b, :VL[b], :].T.reshape(2, 128, VL[b])
        for wi, (w0, wl) in enumerate(WK[b]):
            for e in range(2):
                c0 = cols[f"keysT_b{b}_w{wi}_e{e}"]
                shared[:, c0:c0 + wl] = kT[e, :, w0:w0 + wl]
    for hc in range(2):
        shared[:, cols[f"wz{hc}"] + QSH - 1] = Wv[hc * 128:(hc + 1) * 128]
    for b in range(B):
        for ci in range(NCH[b]):
            c0 = ci * 128
            csz = min(128, VL[b] - c0)
            vcol = cols[f"vals_b{b}_c{ci}"]
            shared[:csz, vcol:vcol + DV] = values[b, c0:c0 + csz, :]
            shared[:csz, vcol + DV] = 1.0
    shared[:, cols["identb"]:cols["identb"] + 128] = np.eye(128)

    in_maps = []
    for c in range(N_CORES):
        blob = shared.copy()
        qsl = queries[:, c * QSH:(c + 1) * QSH, :]  # [B, 16, QS]
        qT = np.ascontiguousarray(qsl.transpose(0, 2, 1))  # [B, QS, 16]
        for e in range(2):
            c0 = cols[f"qT{e}"]
            for b in range(B):
                blob[:, c0 + b * QSH:c0 + (b + 1) * QSH] = \
                    qT[b, e * 128:(e + 1) * 128, :]
        in_maps.append({"blob": blob.astype(bf16)})
    return in_maps


def kernel(queries, keys, values, valid_lens, Wq, Wk, Wv):
    from concourse.bass_utils import run_bass_kernel_spmd

    vl = tuple(int(x) for x in np.asarray(valid_lens).reshape(-1))
    assert len(vl) == B and all(1 <= v <= KMAX for v in vl)

    if vl not in _PROGRAM_CACHE:
        _PROGRAM_CACHE[vl] = _build_program(vl)
    nc = _PROGRAM_CACHE[vl]

    in_maps = _host_prep(queries, keys, values, vl, Wq, Wk, Wv)
    res = run_bass_kernel_spmd(nc, in_maps, list(range(N_CORES)))

    out = np.empty((B, Q, DV), np.float32)
    for c in range(N_CORES):
        # device rows are in PERM order: row c holds q = PERM[c]
        out[:, c * QSH + PERM, :] = res.results[c]["out"]
    return out
